# revision 1
# baseline (speedup 1.0000x reference)
"""DualAxisBlock Trainium2 kernel: time-attention + sparse-T stock-attention +
FFN, sharded over 8 NeuronCores (stocks split 32/core, stock-attn pairs
redistributed via AllToAll).

Self-contained: builds a Bass/Tile SPMD program, shards the full inputs on the
host, runs on cores 0-7, reassembles the full output.
"""
import sys

sys.path.insert(0, "/opt/trn_rl_repo")

import os

import numpy as np

import concourse.bass as bass
import concourse.mybir as mybir
import concourse.tile as tile
from concourse import bacc
from concourse.bass_utils import run_bass_kernel_spmd
from concourse.masks import make_identity

F32 = mybir.dt.float32
AF = mybir.ActivationFunctionType

B, N, T, D = 2, 256, 256, 256
HEADS, HDIM, FF = 8, 32, 1024
EPS = 1e-6
NCORES = 8
NLOC = N // NCORES            # 32 stocks per core
TP = 46                       # active timesteps
PPC = 12                      # pairs per core (padded)
NPAD = NCORES * PPC           # 96 padded global pairs (92 real)
ACT_PIECES = [(0, 8, 30), (240, 1, 16)]   # (start, step, count) of active t
PHASES = os.environ.get("KPH", "ABC")     # debug: subset of phases to emit


def _emit_rmsnorm(nc, K, x, tag):
    """x: [128,512] FM (col=256*dc+t). Returns xn [128,512]. Uses Ln+Exp
    (same ACT table set as softmax exp) instead of sqrt/rsqrt."""
    sq = K.sb.tile([128, 512], F32, tag="sq")
    for dc in range(2):
        nc.vector.tensor_mul(sq[:, 256 * dc:256 * dc + 256],
                             x[:, 256 * dc:256 * dc + 256],
                             x[:, 256 * dc:256 * dc + 256])
    ss = K.ps.tile([128, 256], F32, tag="small", bufs=1)
    for dc in range(2):
        nc.tensor.matmul(ss[:], K.ones[:], sq[:, 256 * dc:256 * dc + 256],
                         start=(dc == 0), stop=(dc == 1))
    lnm = K.sb.tile([128, 256], F32, tag="ln")
    nc.scalar.activation(out=lnm[:], in_=ss[:], func=AF.Ln, scale=1.0 / D,
                         bias=K.epst[:])
    rstd = K.sb.tile([128, 256], F32, tag="rs")
    nc.scalar.activation(out=rstd[:], in_=lnm[:], func=AF.Exp, scale=-0.5)
    xn = K.sb.tile([128, 512], F32, tag="xn")
    for dc in range(2):
        nc.vector.tensor_mul(xn[:, 256 * dc:256 * dc + 256],
                             x[:, 256 * dc:256 * dc + 256], rstd[:])
    return xn


def _emit_proj_fm(nc, K, xn, w, tag, engine):
    """q/k projection: out FM [128,512] (col = 256*oc + t). w: [128,2,256]."""
    ps = K.ps.tile([128, 512], F32, tag="proj", bufs=2)
    for oc in range(2):
        for dc in range(2):
            nc.tensor.matmul(ps[:, 256 * oc:256 * oc + 256],
                             w[:, dc, 128 * oc:128 * oc + 128],
                             xn[:, 256 * dc:256 * dc + 256],
                             start=(dc == 0), stop=(dc == 1))
    out = K.sb.tile([128, 512], F32, tag=tag)
    if engine == "scalar":
        nc.scalar.copy(out=out[:], in_=ps[:])
    else:
        nc.vector.tensor_copy(out[:], ps[:])
    return out


def _emit_proj_tm(nc, K, xn, w, tag):
    """v projection token-major: out [128,512] (col = 256*tokc + dout)."""
    ps = K.ps.tile([128, 512], F32, tag="proj", bufs=2)
    for tc_ in range(2):
        for dc in range(2):
            nc.tensor.matmul(ps[:, 256 * tc_:256 * tc_ + 256],
                             xn[:, 256 * dc + 128 * tc_:256 * dc + 128 * tc_ + 128],
                             w[:, dc, :],
                             start=(dc == 0), stop=(dc == 1))
    out = K.sb.tile([128, 512], F32, tag=tag)
    nc.vector.tensor_copy(out[:], ps[:])
    return out


def _emit_attn_core(nc, K, q, k, v, bias_ap_fn, pair=None):
    """Full softmax attention over 256 tokens, 8 heads.
    q,k FM [128,512]; v TM [128,512]. Returns o_sb FM [128,512] (col=256*g+t,
    head 4g+j at partitions 32j). bias_ap_fn(c) -> [128,1] AP or None.
    pair=(s, o_dst): operands/o use pair layout col = 512*chunk + 256*s + t."""
    if pair is None:
        s_off, o_sb, cstride = 0, None, 256
    else:
        s_off, o_sb = pair[0] * 256, pair[1]
        cstride = 512
    S = K.sb.tile([128, 4096], F32, tag="S", bufs=2)
    Sv = S[:].rearrange("p (h c t) -> p h c t", h=8, c=2)
    Ziv = K.sb.tile([128, 512], F32, tag="Ziv")
    if o_sb is None:
        o_sb = K.sb.tile([128, 512], F32, tag="osb")
    for hg in range(4):               # half-groups: heads 2hg, 2hg+1
        sc = K.ps.tile([128, 1024], F32, tag="sc", bufs=2)
        for j2 in range(2):
            h = 2 * hg + j2
            j = h % 4
            g = h // 4
            for c in range(2):
                qb = cstride * g + s_off
                nc.tensor.matmul(
                    sc[:, 512 * j2 + 256 * c:512 * j2 + 256 * c + 256],
                    k[32 * j:32 * j + 32, qb + 128 * c:qb + 128 * c + 128],
                    q[32 * j:32 * j + 32, qb:qb + 256],
                    start=True, stop=True, tile_position=(32 * j, 0))
        scv = sc[:].rearrange("p (j2 c t) -> p j2 c t", j2=2, c=2)
        if bias_ap_fn is None:
            nc.scalar.activation(out=Sv[:, 2 * hg:2 * hg + 2, :, :], in_=scv[:],
                                 func=AF.Exp)
        else:
            for c in range(2):
                nc.scalar.activation(out=Sv[:, 2 * hg:2 * hg + 2, c, :],
                                     in_=scv[:, :, c, :], func=AF.Exp,
                                     bias=bias_ap_fn(c))
    for g in range(2):
        zb = K.ps.tile([128, 256], F32, tag="small", bufs=1)
        for j in range(4):
            for c in range(2):
                nc.tensor.matmul(zb[32 * j:32 * j + 32, :], K.ones[:, 0:32],
                                 Sv[:, 4 * g + j, c, :],
                                 start=(c == 0), stop=(c == 1),
                                 tile_position=(0, 32 * j))
        nc.vector.reciprocal(out=Ziv[:, 256 * g:256 * g + 256], in_=zb[:])
        op = K.ps.tile([128, 256], F32, tag="ops")
        for j in range(4):
            for c in range(2):
                h = 4 * g + j
                nc.tensor.matmul(op[32 * j:32 * j + 32, :],
                                 v[:, 256 * c + 32 * h:256 * c + 32 * h + 32],
                                 Sv[:, h, c, :],
                                 start=(c == 0), stop=(c == 1),
                                 tile_position=(0, 32 * j))
        nc.vector.tensor_mul(o_sb[:, cstride * g + s_off:cstride * g + s_off + 256],
                             op[:], Ziv[:, 256 * g:256 * g + 256])
    return o_sb


def _emit_wo_resid(nc, K, o_sb, wo, x, tag):
    """h = x + o @ Wo. Returns h [128,512] FM."""
    ps = K.ps.tile([128, 512], F32, tag="proj", bufs=2)
    for oc in range(2):
        for dmc in range(2):
            nc.tensor.matmul(ps[:, 256 * oc:256 * oc + 256],
                             wo[:, dmc, 128 * oc:128 * oc + 128],
                             o_sb[:, 256 * dmc:256 * dmc + 256],
                             start=(dmc == 0), stop=(dmc == 1))
    h = K.sb.tile([128, 512], F32, tag=tag)
    for c in range(2):
        nc.vector.tensor_add(h[:, 256 * c:256 * c + 256],
                             x[:, 256 * c:256 * c + 256],
                             ps[:, 256 * c:256 * c + 256])
    return h


def _emit_fm_to_tm(nc, K, src, n_cols_pieces, dst, dst_row_of_piece):
    """PE-transpose FM [128, cols] pieces -> TM rows in dst.
    n_cols_pieces: list of (src_col_ap, count). dst rows at dst_row_of_piece."""
    for dc in range(2):
        for (col_ap, cnt), row0 in zip(n_cols_pieces(dc), dst_row_of_piece):
            tp = K.ps.tile([128, 128], F32, tag="small", bufs=1)
            nc.tensor.transpose(tp[0:cnt, :], col_ap, K.ident[:])
            nc.vector.tensor_copy(dst[row0:row0 + cnt, 128 * dc:128 * dc + 128],
                                  tp[0:cnt, :])


class _K:
    pass


def build(nc):
    hfm = nc.declare_dram_parameter("hfm", [B, NLOC, D, T], F32, isOutput=False)
    biasP = nc.declare_dram_parameter("biasP", [PPC, 2, 128], F32, isOutput=False)
    wnames = ["wq_t", "wk_t", "wv_t", "wo_t", "wq_s", "wk_s", "wv_s", "wo_s"]
    wext = {w: nc.declare_dram_parameter(w, [D, D], F32, isOutput=False)
            for w in wnames}
    w1e = nc.declare_dram_parameter("w1", [D, FF], F32, isOutput=False)
    w2e = nc.declare_dram_parameter("w2", [FF, D], F32, isOutput=False)
    b1e = nc.declare_dram_parameter("b1", [FF], F32, isOutput=False)
    b2e = nc.declare_dram_parameter("b2", [D], F32, isOutput=False)
    outfm = nc.declare_dram_parameter("outfm", [B, NLOC, D, T], F32, isOutput=True)

    K = _K()
    with tile.TileContext(nc) as tc:
        import contextlib
        with contextlib.ExitStack() as ctx:
            K.sb = ctx.enter_context(tc.tile_pool(name="sb", bufs=2))
            K.single = ctx.enter_context(tc.tile_pool(name="single", bufs=1))
            K.ps = ctx.enter_context(tc.tile_pool(name="ps", bufs=1, space="PSUM"))
            dram = ctx.enter_context(tc.tile_pool(name="dram", bufs=1, space="DRAM"))

            h1d = dram.tile([B, NLOC, D, T], F32, tag="h1d")
            xnd = dram.tile([B, NLOC, D, T], F32, tag="xnd")
            snd1 = dram.tile([NPAD, NLOC, D], F32, tag="snd1")
            rcv1 = dram.tile([NPAD, NLOC, D], F32, tag="rcv1")
            snd2 = dram.tile([NPAD, NLOC, D], F32, tag="snd2")
            rcv2 = dram.tile([NPAD, NLOC, D], F32, tag="rcv2")

            # ---- constants / weights to SBUF ----
            K.ones = K.single.tile([128, 128], F32, tag="ones")
            nc.gpsimd.memset(K.ones[:], 1.0)
            K.ident = K.single.tile([128, 128], F32, tag="ident")
            make_identity(nc, K.ident[:])
            zer = K.single.tile([128, D], F32, tag="zer")
            nc.gpsimd.memset(zer[:], 0.0)
            K.epst = K.single.tile([128, 1], F32, tag="epst")
            nc.gpsimd.memset(K.epst[:], EPS)

            wsb = {}
            for w in wnames:
                wsb[w] = K.single.tile([128, 2, D], F32, tag=w, name=w)
                nc.sync.dma_start(out=wsb[w][:],
                                  in_=wext[w][:].rearrange("(c p) x -> p c x", p=128))
            w1 = K.single.tile([128, 2, FF], F32, tag="w1")
            nc.sync.dma_start(out=w1[:], in_=w1e[:].rearrange("(c p) x -> p c x", p=128))
            w2 = K.single.tile([128, 8, D], F32, tag="w2")
            nc.sync.dma_start(out=w2[:], in_=w2e[:].rearrange("(c p) x -> p c x", p=128))
            b1s = K.single.tile([128, 8], F32, tag="b1s")
            nc.sync.dma_start(out=b1s[:], in_=b1e[:].rearrange("(c p) -> p c", p=128))
            b2s = K.single.tile([128, 2], F32, tag="b2s")
            nc.sync.dma_start(out=b2s[:], in_=b2e[:].rearrange("(c p) -> p c", p=128))
            bps = K.single.tile([128, PPC, 2], F32, tag="bps")
            nc.sync.dma_start(out=bps[:], in_=biasP[:].rearrange("q c p -> p q c"))

            # zero the pad rows of snd1 (global pairs 92..95)
            for pad in range(B * TP, NPAD):
                nc.sync.dma_start(out=snd1[pad], in_=zer[0:NLOC, :])

            # ================= PHASE A: time attention (2-stage pipe) ====
            def a_stage1(b, n):
                x = K.sb.tile([128, 512], F32, tag="x")
                nc.sync.dma_start(
                    out=x[:].rearrange("p (c t) -> p c t", c=2),
                    in_=hfm[b, n].rearrange("(c p) t -> p c t", p=128))
                xn = _emit_rmsnorm(nc, K, x, "a")
                q = _emit_proj_fm(nc, K, xn, wsb["wq_t"], "q", "vector")
                k = _emit_proj_fm(nc, K, xn, wsb["wk_t"], "k", "vector")
                v = _emit_proj_tm(nc, K, xn, wsb["wv_t"], "v")
                return (b, n, x, q, k, v)

            def a_stage2(st):
                b, n, x, q, k, v = st
                o = _emit_attn_core(nc, K, q, k, v, None)
                h1 = _emit_wo_resid(nc, K, o, wsb["wo_t"], x, "h1")
                nc.sync.dma_start(
                    out=h1d[b, n].rearrange("(c p) t -> p c t", p=128),
                    in_=h1[:].rearrange("p (c t) -> p c t", c=2))
                h1a = K.sb.tile([128, 256], F32, tag="h1a")
                _emit_fm_to_tm(
                    nc, K,
                    h1,
                    lambda dc: [(h1[:, 256 * dc:256 * dc + 240:8], 30),
                                (h1[:, 256 * dc + 240:256 * dc + 256], 16)],
                    h1a, [0, 32])
                nc.sync.dma_start(out=snd1[b * TP:b * TP + 30, n, :],
                                  in_=h1a[0:30, :])
                nc.sync.dma_start(out=snd1[b * TP + 30:(b + 1) * TP, n, :],
                                  in_=h1a[32:48, :])
                if "C" not in PHASES:
                    nc.sync.dma_start(
                        out=outfm[b, n].rearrange("(c p) t -> p c t", p=128),
                        in_=h1[:].rearrange("p (c t) -> p c t", c=2))

            prev = None
            for b in range(B):
                for n in range(NLOC):
                    cur = a_stage1(b, n)
                    if prev is not None:
                        a_stage2(prev)
                    prev = cur
            if prev is not None:
                a_stage2(prev)

            if "B" in PHASES:
                nc.gpsimd.collective_compute(
                    "AllToAll", mybir.AluOpType.bypass,
                    replica_groups=[list(range(NCORES))],
                    ins=[snd1.opt()], outs=[rcv1.opt()])

            # ================= PHASE B: stock attention =================
            rcv1v = rcv1[:].rearrange("(sg q) s d -> sg q s d", q=PPC)
            snd2v = snd2[:].rearrange("(sg q) s d -> sg q s d", q=PPC)
            def b_stage1(p):
                xs_tm = K.sb.tile([128, 512], F32, tag="xstm")
                for sg in range(8):
                    nc.sync.dma_start(
                        out=xs_tm[32 * (sg % 4):32 * (sg % 4) + 32,
                                  256 * (sg // 4):256 * (sg // 4) + 256],
                        in_=rcv1v[sg, p, :, :])
                xsf = K.sb.tile([128, 512], F32, tag="xsf")
                for dc in range(2):
                    for sc_ in range(2):
                        tp = K.ps.tile([128, 128], F32, tag="small", bufs=1)
                        nc.tensor.transpose(
                            tp[:], xs_tm[:, 256 * sc_ + 128 * dc:256 * sc_ + 128 * dc + 128],
                            K.ident[:])
                        nc.vector.tensor_copy(
                            xsf[:, 256 * dc + 128 * sc_:256 * dc + 128 * sc_ + 128], tp[:])
                xn = _emit_rmsnorm(nc, K, xsf, "b")
                q = _emit_proj_fm(nc, K, xn, wsb["wq_s"], "q", "vector")
                k = _emit_proj_fm(nc, K, xn, wsb["wk_s"], "k", "vector")
                v = _emit_proj_tm(nc, K, xn, wsb["wv_s"], "v")
                return (p, xsf, q, k, v)

            def b_stage2(st):
                p, xsf, q, k, v = st
                o = _emit_attn_core(nc, K, q, k, v,
                                    lambda c: bps[:, p, c:c + 1])
                h2 = _emit_wo_resid(nc, K, o, wsb["wo_s"], xsf, "h1")
                h2t = K.sb.tile([128, 512], F32, tag="h2t")
                for dc in range(2):
                    for sc_ in range(2):
                        tp = K.ps.tile([128, 128], F32, tag="small", bufs=1)
                        nc.tensor.transpose(
                            tp[:], h2[:, 256 * dc + 128 * sc_:256 * dc + 128 * sc_ + 128],
                            K.ident[:])
                        nc.vector.tensor_copy(
                            h2t[:, 256 * sc_ + 128 * dc:256 * sc_ + 128 * dc + 128], tp[:])
                for sg in range(8):
                    nc.sync.dma_start(
                        out=snd2v[sg, p, :, :],
                        in_=h2t[32 * (sg % 4):32 * (sg % 4) + 32,
                                256 * (sg // 4):256 * (sg // 4) + 256])

            prevb = None
            for p in range(PPC if "B" in PHASES else 0):
                curb = b_stage1(p)
                if prevb is not None:
                    b_stage2(prevb)
                prevb = curb
            if prevb is not None:
                b_stage2(prevb)

            if "B" in PHASES:
                nc.gpsimd.collective_compute(
                    "AllToAll", mybir.AluOpType.bypass,
                    replica_groups=[list(range(NCORES))],
                    ins=[snd2.opt()], outs=[rcv2.opt()])

            # ================= PHASE C1: merge corrections + rmsnorm ====
            for b in range(B if "C" in PHASES else 0):
                for n in range(NLOC):
                    h1 = K.sb.tile([128, 512], F32, tag="x")
                    nc.sync.dma_start(
                        out=h1[:].rearrange("p (c t) -> p c t", c=2),
                        in_=h1d[b, n].rearrange("(c p) t -> p c t", p=128))
                    if "B" in PHASES:
                        corr = K.sb.tile([128, 256], F32, tag="corr")
                        nc.sync.dma_start(out=corr[0:TP, :],
                                          in_=rcv2[b * TP:(b + 1) * TP, n, :])
                        for dc in range(2):
                            tp = K.ps.tile([128, 128], F32, tag="small", bufs=1)
                            nc.tensor.transpose(tp[:, 0:TP],
                                                corr[0:TP, 128 * dc:128 * dc + 128],
                                                K.ident[0:TP, 0:TP])
                            nc.vector.tensor_copy(h1[:, 256 * dc:256 * dc + 240:8],
                                                  tp[:, 0:30])
                            nc.vector.tensor_copy(h1[:, 256 * dc + 240:256 * dc + 256],
                                                  tp[:, 30:TP])
                    xn2 = _emit_rmsnorm(nc, K, h1, "c")
                    nc.sync.dma_start(
                        out=h1d[b, n].rearrange("(c p) t -> p c t", p=128),
                        in_=h1[:].rearrange("p (c t) -> p c t", c=2))
                    nc.sync.dma_start(
                        out=xnd[b, n].rearrange("(c p) t -> p c t", p=128),
                        in_=xn2[:].rearrange("p (c t) -> p c t", c=2))

            # ================= PHASE C2: FFN (2 seqs per iteration) ======
            seqs = [(b, n) for b in range(B) for n in range(NLOC)]
            for i0 in range(0, len(seqs) if "C" in PHASES else 0, 2):
                pair = seqs[i0:i0 + 2]
                # xn2p/h1p: [128, 1024], col = 512*dc + 256*s + t
                xn2p = K.sb.tile([128, 1024], F32, tag="xn2")
                h1p = K.sb.tile([128, 1024], F32, tag="h1p")
                for s, (b, n) in enumerate(pair):
                    for dc in range(2):
                        nc.sync.dma_start(
                            out=xn2p[:, 512 * dc + 256 * s:512 * dc + 256 * s + 256],
                            in_=xnd[b, n][128 * dc:128 * dc + 128, :])
                        nc.sync.dma_start(
                            out=h1p[:, 512 * dc + 256 * s:512 * dc + 256 * s + 256],
                            in_=h1d[b, n][128 * dc:128 * dc + 128, :])
                # gsb: [128, 4096], col = 512*fc + 256*s + t
                gsb = K.sb.tile([128, 4096], F32, tag="gsb", bufs=1)
                for fp_ in range(4):
                    fps = K.ps.tile([128, 1024], F32, tag="sc", bufs=2)
                    for sub in range(2):
                        fc = 2 * fp_ + sub
                        for dc in range(2):
                            nc.tensor.matmul(
                                fps[:, 512 * sub:512 * sub + 512],
                                w1[:, dc, 128 * fc:128 * fc + 128],
                                xn2p[:, 512 * dc:512 * dc + 512],
                                start=(dc == 0), stop=(dc == 1))
                    for sub in range(2):
                        fc = 2 * fp_ + sub
                        nc.scalar.activation(
                            out=gsb[:, 512 * fc:512 * fc + 512],
                            in_=fps[:, 512 * sub:512 * sub + 512],
                            func=AF.Gelu_apprx_tanh, bias=b1s[:, fc:fc + 1])
                # ffo: [128, 1024], col = 512*oc + 256*s + t
                ffo = K.ps.tile([128, 1024], F32, tag="sc", bufs=2)
                for oc in range(2):
                    for fc in range(8):
                        nc.tensor.matmul(ffo[:, 512 * oc:512 * oc + 512],
                                         w2[:, fc, 128 * oc:128 * oc + 128],
                                         gsb[:, 512 * fc:512 * fc + 512],
                                         start=(fc == 0), stop=(fc == 7))
                fin = K.sb.tile([128, 1024], F32, tag="fin")
                for dc in range(2):
                    nc.vector.scalar_tensor_tensor(
                        out=fin[:, 512 * dc:512 * dc + 512],
                        in0=ffo[:, 512 * dc:512 * dc + 512],
                        scalar=b2s[:, dc:dc + 1],
                        in1=h1p[:, 512 * dc:512 * dc + 512],
                        op0=mybir.AluOpType.add, op1=mybir.AluOpType.add)
                for s, (b, n) in enumerate(pair):
                    for dc in range(2):
                        nc.sync.dma_start(
                            out=outfm[b, n][128 * dc:128 * dc + 128, :],
                            in_=fin[:, 512 * dc + 256 * s:512 * dc + 256 * s + 256])
    nc.compile()
    return nc


_CACHED = None


def _get_nc():
    global _CACHED
    if _CACHED is None:
        nc = bacc.Bacc("TRN2", target_bir_lowering=False, debug=False,
                       num_devices=NCORES)
        _CACHED = build(nc)
    return _CACHED


def _host_inputs(h, stock_mask, norm_t_w, norm_s_w, norm_ff_w,
                 Wq_t, Wk_t, Wv_t, Wo_t, Wq_s, Wk_s, Wv_s, Wo_s,
                 W1, b1, W2, b2):
    f = np.float32
    sc = 1.0 / np.sqrt(HDIM)
    wq_t = (norm_t_w[:, None] * Wq_t * sc).astype(f)
    wk_t = (norm_t_w[:, None] * Wk_t).astype(f)
    wv_t = (norm_t_w[:, None] * Wv_t).astype(f)
    wq_s = (norm_s_w[:, None] * Wq_s * sc).astype(f)
    wk_s = (norm_s_w[:, None] * Wk_s).astype(f)
    wv_s = (norm_s_w[:, None] * Wv_s).astype(f)
    w1 = (norm_ff_w[:, None] * W1).astype(f)
    mask_bias = ((stock_mask.astype(np.float32) - 1.0) * 1e4).astype(f)  # [B,N]
    in_maps = []
    for i in range(NCORES):
        hfm = np.ascontiguousarray(
            h[:, i * NLOC:(i + 1) * NLOC].transpose(0, 1, 3, 2)).astype(f)
        bP = np.zeros((PPC, 2, 128), f)
        for p in range(PPC):
            gp = PPC * i + p
            bb = min(gp // TP, B - 1)
            bP[p] = mask_bias[bb].reshape(2, 128)
        in_maps.append({
            "hfm": hfm, "biasP": bP,
            "wq_t": wq_t, "wk_t": wk_t, "wv_t": wv_t,
            "wo_t": Wo_t.astype(f),
            "wq_s": wq_s, "wk_s": wk_s, "wv_s": wv_s,
            "wo_s": Wo_s.astype(f),
            "w1": w1, "w2": W2.astype(f),
            "b1": b1.astype(f), "b2": b2.astype(f),
        })
    return in_maps


def kernel(**inputs):
    inputs = {k: np.asarray(v) for k, v in inputs.items()}
    nc = _get_nc()
    in_maps = _host_inputs(**inputs)
    res = run_bass_kernel_spmd(nc, in_maps, list(range(NCORES)))
    out = np.empty((B, N, T, D), np.float32)
    for i in range(NCORES):
        out[:, i * NLOC:(i + 1) * NLOC] = \
            res.results[i]["outfm"].transpose(0, 1, 3, 2)
    return out


if __name__ == "__main__":
    rng = np.random.default_rng(0)
    h = rng.normal(size=(B, N, T, D)).astype(np.float32)
    out = kernel(
        h=h, stock_mask=np.ones((B, N), np.int32),
        norm_t_w=np.ones(D, np.float32), norm_s_w=np.ones(D, np.float32),
        norm_ff_w=np.ones(D, np.float32),
        Wq_t=rng.normal(size=(D, D)).astype(np.float32) * 0.02,
        Wk_t=rng.normal(size=(D, D)).astype(np.float32) * 0.02,
        Wv_t=rng.normal(size=(D, D)).astype(np.float32) * 0.02,
        Wo_t=rng.normal(size=(D, D)).astype(np.float32) * 0.02,
        Wq_s=rng.normal(size=(D, D)).astype(np.float32) * 0.02,
        Wk_s=rng.normal(size=(D, D)).astype(np.float32) * 0.02,
        Wv_s=rng.normal(size=(D, D)).astype(np.float32) * 0.02,
        Wo_s=rng.normal(size=(D, D)).astype(np.float32) * 0.02,
        W1=rng.normal(size=(D, FF)).astype(np.float32) * 0.02,
        b1=np.zeros(FF, np.float32),
        W2=rng.normal(size=(FF, D)).astype(np.float32) * 0.02,
        b2=np.zeros(D, np.float32),
    )
    print("out", out.shape, out.dtype, np.abs(out).max())



# revision 11
# speedup vs baseline: 2.0918x; 2.0918x over previous
"""DualAxisBlock Trainium2 kernel: time-attention + sparse-T stock-attention +
FFN, sharded over 8 NeuronCores (stocks split 32/core, stock-attn pairs
redistributed via b-split AllToAlls).

v2: bf16 matmul datapath (4x PE), DVE Newton-rsqrt rmsnorm (no act-table
thrash), fast-approx reciprocal, SBUF-resident residual stream (no DRAM
round trip between phases), DMA-transpose for stock-attn layout changes.

Self-contained: builds a Bass/Tile SPMD program, shards the full inputs on the
host, runs on cores 0-7, reassembles the full output.
"""
import sys

sys.path.insert(0, "/opt/trn_rl_repo")

import os

import numpy as np

import concourse.bass as bass
import concourse.mybir as mybir
import concourse.tile as tile
from concourse import bacc
from concourse.bass_utils import run_bass_kernel_spmd
from concourse.masks import make_identity

F32 = mybir.dt.float32
BF = mybir.dt.bfloat16
U32 = mybir.dt.uint32
AF = mybir.ActivationFunctionType
ALU = mybir.AluOpType

B, N, T, D = 2, 256, 256, 256
HEADS, HDIM, FF = 8, 32, 1024
NCORES = 8
NLOC = N // NCORES            # 32 stocks per core
TP = 46                       # active timesteps per batch
TPP = 48                      # padded to 48 (6 rows per core chunk)
PPC = TPP // NCORES           # 6 pairs per core per batch block
RSQRT_MAGIC = 0x5F3759DF
PHASES = os.environ.get("KPH", "ABC")     # debug: subset of phases to emit


def _emit_rmsnorm(nc, K, x, tag):
    """x: [128,512] bf16 FM (col=256*dc+t). Returns xn bf16 [128,512].
    rstd via fast-inverse-sqrt seed + 1 Newton iteration (DVE+Pool only;
    avoids scalar-engine act-table loads)."""
    sq = K.sb.tile([128, 512], BF, tag="sq")
    nc.vector.tensor_mul(sq[:], x[:], x[:])
    ss = K.ps.tile([128, 256], F32, tag="small", bufs=1)
    for dc in range(2):
        nc.tensor.matmul(ss[:], K.ones_sc[:], sq[:, 256 * dc:256 * dc + 256],
                         start=(dc == 0), stop=(dc == 1))
    # ss = mean(x^2) per token (broadcast over partitions)
    u = K.sb.tile([128, 256], U32, tag="nru")
    nc.vector.tensor_scalar(u[:], ss[:].bitcast(U32), 1, None,
                            ALU.logical_shift_right)
    y0 = K.sb.tile([128, 256], F32, tag="nry")
    nc.gpsimd.tensor_tensor(out=y0[:].bitcast(U32), in0=K.magic[:], in1=u[:],
                            op=ALU.subtract)
    a = K.sb.tile([128, 256], F32, tag="nra")
    nc.gpsimd.tensor_mul(a[:], y0[:], y0[:])
    bb = K.sb.tile([128, 256], F32, tag="nrb")
    nc.vector.tensor_mul(bb[:], ss[:], a[:])
    dd = K.sb.tile([128, 256], F32, tag="nrd")
    nc.gpsimd.tensor_scalar(dd[:], bb[:], -0.5, 1.5, ALU.mult, ALU.add)
    rstd = K.sb.tile([128, 256], BF, tag="rstd")
    nc.gpsimd.tensor_mul(rstd[:], y0[:], dd[:])
    xn = K.sb.tile([128, 512], BF, tag="xn")
    for dc in range(2):
        nc.vector.tensor_mul(xn[:, 256 * dc:256 * dc + 256],
                             x[:, 256 * dc:256 * dc + 256], rstd[:])
    return xn


def _emit_proj_fm(nc, K, xn, w, tag, engine):
    """q/k projection: out bf16 FM [128,512] (col = 256*oc + t)."""
    ps = K.ps.tile([128, 512], F32, tag="proj", bufs=2)
    for oc in range(2):
        for dc in range(2):
            nc.tensor.matmul(ps[:, 256 * oc:256 * oc + 256],
                             w[:, dc, 128 * oc:128 * oc + 128],
                             xn[:, 256 * dc:256 * dc + 256],
                             start=(dc == 0), stop=(dc == 1))
    out = K.sb.tile([128, 512], BF, tag=tag, bufs=4)
    if engine == "scalar":
        nc.scalar.copy(out=out[:], in_=ps[:])
    elif engine == "gpsimd":
        nc.gpsimd.tensor_copy(out[:], ps[:])
    else:
        nc.vector.tensor_copy(out[:], ps[:])
    return out


def _emit_proj_tm(nc, K, xn, w, tag, engine):
    """v projection token-major: out bf16 [128,512] (col = 256*tokc + dout)."""
    ps = K.ps.tile([128, 512], F32, tag="proj", bufs=2)
    for tc_ in range(2):
        for dc in range(2):
            nc.tensor.matmul(ps[:, 256 * tc_:256 * tc_ + 256],
                             xn[:, 256 * dc + 128 * tc_:256 * dc + 128 * tc_ + 128],
                             w[:, dc, :],
                             start=(dc == 0), stop=(dc == 1))
    out = K.sb.tile([128, 512], BF, tag=tag, bufs=4)
    if engine == "scalar":
        nc.scalar.copy(out=out[:], in_=ps[:])
    elif engine == "gpsimd":
        nc.gpsimd.tensor_copy(out[:], ps[:])
    else:
        nc.vector.tensor_copy(out[:], ps[:])
    return out


def _emit_attn_core(nc, K, q, k, v, bias_ap_fn):
    """Full softmax attention over 256 tokens, 8 heads, bf16 datapath.
    q,k FM bf16 [128,512]; v TM bf16 [128,512]. Returns o_sb bf16 [128,512]
    (col=256*g+t, head 4g+j at partitions 32j). bias_ap_fn(c)->[128,1]|None."""
    S = K.sb.tile([128, 4096], BF, tag="S", bufs=2)
    Sv = S[:].rearrange("p (h c t) -> p h c t", h=8, c=2)
    Ziv = K.sb.tile([128, 512], F32, tag="Ziv")
    o_sb = K.sb.tile([128, 512], BF, tag="osb")
    for hg in range(4):               # half-groups: heads 2hg, 2hg+1
        sc = K.ps.tile([128, 1024], F32, tag="sc", bufs=2)
        for j2 in range(2):
            h = 2 * hg + j2
            j = h % 4
            g = h // 4
            for c in range(2):
                qb = 256 * g
                nc.tensor.matmul(
                    sc[:, 512 * j2 + 256 * c:512 * j2 + 256 * c + 256],
                    k[32 * j:32 * j + 32, qb + 128 * c:qb + 128 * c + 128],
                    q[32 * j:32 * j + 32, qb:qb + 256],
                    start=True, stop=True, tile_position=(32 * j, 0))
        if bias_ap_fn is None:
            nc.scalar.activation(out=S[:, 1024 * hg:1024 * hg + 1024],
                                 in_=sc[:], func=AF.Exp)
        else:
            scv = sc[:].rearrange("p (j2 c t) -> p j2 c t", j2=2, c=2)
            for c in range(2):
                nc.scalar.activation(out=Sv[:, 2 * hg:2 * hg + 2, c, :],
                                     in_=scv[:, :, c, :], func=AF.Exp,
                                     bias=bias_ap_fn(c))
    for g in range(2):
        zb = K.ps.tile([128, 256], F32, tag="small", bufs=1)
        for j in range(4):
            for c in range(2):
                nc.tensor.matmul(zb[32 * j:32 * j + 32, :], K.ones[:, 0:32],
                                 Sv[:, 4 * g + j, c, :],
                                 start=(c == 0), stop=(c == 1),
                                 tile_position=(0, 32 * j))
        nc.vector.reciprocal_approx_fast(out=Ziv[:, 256 * g:256 * g + 256],
                                         in_=zb[:])
        op = K.ps.tile([128, 256], F32, tag="ops")
        for j in range(4):
            for c in range(2):
                h = 4 * g + j
                nc.tensor.matmul(op[32 * j:32 * j + 32, :],
                                 v[:, 256 * c + 32 * h:256 * c + 32 * h + 32],
                                 Sv[:, h, c, :],
                                 start=(c == 0), stop=(c == 1),
                                 tile_position=(0, 32 * j))
        nc.vector.tensor_mul(o_sb[:, 256 * g:256 * g + 256],
                             op[:], Ziv[:, 256 * g:256 * g + 256])
    return o_sb


def _emit_wo_resid(nc, K, o_sb, wo, x, h_out):
    """h_out[:] = x + o @ Wo (bf16)."""
    ps = K.ps.tile([128, 512], F32, tag="proj", bufs=2)
    for oc in range(2):
        for dmc in range(2):
            nc.tensor.matmul(ps[:, 256 * oc:256 * oc + 256],
                             wo[:, dmc, 128 * oc:128 * oc + 128],
                             o_sb[:, 256 * dmc:256 * dmc + 256],
                             start=(dmc == 0), stop=(dmc == 1))
    for c in range(2):
        nc.vector.tensor_add(h_out[:, 256 * c:256 * c + 256],
                             x[:, 256 * c:256 * c + 256],
                             ps[:, 256 * c:256 * c + 256])


class _K:
    pass


def build(nc):
    hfm = nc.declare_dram_parameter("hfm", [B, NLOC, D, T], BF, isOutput=False)
    biasP = nc.declare_dram_parameter("biasP", [B, 2, 128], F32, isOutput=False)
    wnames = ["wq_t", "wk_t", "wv_t", "wo_t", "wq_s", "wk_s", "wv_s", "wo_s"]
    wext = {w: nc.declare_dram_parameter(w, [D, D], BF, isOutput=False)
            for w in wnames}
    w1e = nc.declare_dram_parameter("w1", [D, FF], BF, isOutput=False)
    w2e = nc.declare_dram_parameter("w2", [FF, D], BF, isOutput=False)
    b1e = nc.declare_dram_parameter("b1", [FF], F32, isOutput=False)
    outfm = nc.declare_dram_parameter("outfm", [B, NLOC, D, T], BF, isOutput=True)

    K = _K()
    with tile.TileContext(nc) as tc:
        import contextlib
        with contextlib.ExitStack() as ctx:
            K.sb = ctx.enter_context(tc.tile_pool(name="sb", bufs=2))
            K.single = ctx.enter_context(tc.tile_pool(name="single", bufs=1))
            K.ps = ctx.enter_context(tc.tile_pool(name="ps", bufs=1, space="PSUM"))
            dram = ctx.enter_context(tc.tile_pool(name="dram", bufs=1, space="DRAM"))

            snd1 = [dram.tile([TPP, NLOC, D], BF, tag=f"snd1{b}", name=f"snd1{b}")
                    for b in range(B)]
            rcv1 = [dram.tile([TPP, NLOC, D], BF, tag=f"rcv1{b}", name=f"rcv1{b}")
                    for b in range(B)]
            snd2 = [dram.tile([TPP, NLOC, D], BF, tag=f"snd2{b}", name=f"snd2{b}")
                    for b in range(B)]
            rcv2 = [dram.tile([TPP, NLOC, D], BF, tag=f"rcv2{b}", name=f"rcv2{b}")
                    for b in range(B)]

            # ---- constants / weights to SBUF ----
            K.ones = K.single.tile([128, 128], BF, tag="ones")
            nc.gpsimd.memset(K.ones[:], 1.0)
            K.ones_sc = K.single.tile([128, 128], BF, tag="ones_sc")
            nc.gpsimd.memset(K.ones_sc[:], 1.0 / D)
            K.ident = K.single.tile([128, 128], BF, tag="ident")
            make_identity(nc, K.ident[:])
            K.magic = K.single.tile([128, 256], U32, tag="magic")
            nc.gpsimd.memset(K.magic[:], RSQRT_MAGIC)
            zer = K.single.tile([128, D], BF, tag="zer")
            nc.gpsimd.memset(zer[:], 0.0)

            wsb = {}
            for w in wnames:
                wsb[w] = K.single.tile([128, 2, D], BF, tag=w, name=w)
                nc.sync.dma_start(out=wsb[w][:],
                                  in_=wext[w][:].rearrange("(c p) x -> p c x", p=128))
            w1 = K.single.tile([128, 2, FF], BF, tag="w1")
            nc.sync.dma_start(out=w1[:], in_=w1e[:].rearrange("(c p) x -> p c x", p=128))
            w2 = K.single.tile([128, 8, D], BF, tag="w2")
            nc.sync.dma_start(out=w2[:], in_=w2e[:].rearrange("(c p) x -> p c x", p=128))
            b1s = K.single.tile([128, 8], F32, tag="b1s")
            nc.sync.dma_start(out=b1s[:], in_=b1e[:].rearrange("(c p) -> p c", p=128))
            bps = K.single.tile([128, B, 2], F32, tag="bps")
            nc.sync.dma_start(out=bps[:], in_=biasP[:].rearrange("b c p -> p b c"))

            # persistent residual stream: one bf16 tile per local sequence
            hst = [K.single.tile([128, 512], BF, tag=f"h{i}", name=f"h{i}")
                   for i in range(B * NLOC)]

            # zero the pad rows (t rows 46,47) of each snd1 block
            for b in range(B):
                for pad in range(TP, TPP):
                    nc.sync.dma_start(out=snd1[b][pad], in_=zer[0:NLOC, :])

            # ================= PHASE A: time attention (3-stage pipe) ====
            def a_stage1(b, n):
                x = K.sb.tile([128, 512], BF, tag="x", bufs=4)
                dmae = nc.sync if n % 2 == 0 else nc.scalar
                dmae.dma_start(
                    out=x[:].rearrange("p (c t) -> p c t", c=2),
                    in_=hfm[b, n].rearrange("(c p) t -> p c t", p=128))
                xn = _emit_rmsnorm(nc, K, x, "a")
                q = _emit_proj_fm(nc, K, xn, wsb["wq_t"], "q", "scalar")
                k = _emit_proj_fm(nc, K, xn, wsb["wk_t"], "k", "scalar")
                v = _emit_proj_tm(nc, K, xn, wsb["wv_t"], "v", "vector")
                return (b, n, x, q, k, v)

            def a_stage2(st):
                b, n, x, q, k, v = st
                hti = hst[b * NLOC + n]
                o = _emit_attn_core(nc, K, q, k, v, None)
                _emit_wo_resid(nc, K, o, wsb["wo_t"], x, hti)
                # extract active-t columns token-major for the AllToAll
                tp = K.ps.tile([128, 128], BF, tag="small", bufs=1)
                h1a = K.sb.tile([128, 256], BF, tag="h1a")
                for dc in range(2):
                    nc.tensor.transpose(tp[64 * dc:64 * dc + 30, :],
                                        hti[:, 256 * dc:256 * dc + 240:8],
                                        K.ident[:], tile_position=(0, 64 * dc))
                    nc.tensor.transpose(tp[64 * dc + 32:64 * dc + 48, :],
                                        hti[:, 256 * dc + 240:256 * dc + 256],
                                        K.ident[:], tile_position=(0, 64 * dc + 32))
                    nc.vector.tensor_copy(h1a[0:30, 128 * dc:128 * dc + 128],
                                          tp[64 * dc:64 * dc + 30, :])
                    nc.vector.tensor_copy(h1a[32:48, 128 * dc:128 * dc + 128],
                                          tp[64 * dc + 32:64 * dc + 48, :])
                nc.sync.dma_start(out=snd1[b][0:30, n, :], in_=h1a[0:30, :])
                nc.scalar.dma_start(out=snd1[b][30:TP, n, :], in_=h1a[32:48, :])
                if "C" not in PHASES:
                    nc.sync.dma_start(
                        out=outfm[b, n].rearrange("(c p) t -> p c t", p=128),
                        in_=hti[:].rearrange("p (c t) -> p c t", c=2))

            pend = []
            for b in range(B):
                for n in range(NLOC):
                    pend.append(a_stage1(b, n))
                    if len(pend) > 2:
                        a_stage2(pend.pop(0))
                if "B" in PHASES:
                    while pend:
                        a_stage2(pend.pop(0))
                    nc.gpsimd.collective_compute(
                        "AllToAll", mybir.AluOpType.bypass,
                        replica_groups=[list(range(NCORES))],
                        ins=[snd1[b].opt()], outs=[rcv1[b].opt()])
            while pend:
                a_stage2(pend.pop(0))

            # ================= PHASE B: stock attention =================
            # local pair (b, r): global t-row = PPC*core + r; rcv1[b] rows
            # {6j + r} hold stocks [32j,32j+32) token-major.
            def b_stage1(b, r):
                xsf = K.sb.tile([128, 2, 256], BF, tag="xsf", bufs=4)
                for j in range(NCORES):
                    dmae = nc.sync if j % 2 == 0 else nc.scalar
                    dmae.dma_start(
                        out=xsf[:, :, 32 * j:32 * j + 32],
                        in_=rcv1[b][PPC * j + r], transpose=True)
                xsfv = xsf[:].rearrange("p c s -> p (c s)")
                xn = _emit_rmsnorm(nc, K, xsfv, "b")
                q = _emit_proj_fm(nc, K, xn, wsb["wq_s"], "q", "scalar")
                k = _emit_proj_fm(nc, K, xn, wsb["wk_s"], "k", "scalar")
                v = _emit_proj_tm(nc, K, xn, wsb["wv_s"], "v", "vector")
                return (b, r, xsfv, q, k, v)

            def b_stage2(st):
                b, r, xsf, q, k, v = st
                o = _emit_attn_core(nc, K, q, k, v,
                                    lambda c: bps[:, b, c:c + 1])
                h2 = K.sb.tile([128, 512], BF, tag="h2")
                _emit_wo_resid(nc, K, o, wsb["wo_s"], xsf, h2)
                h2t = K.sb.tile([128, 512], BF, tag="h2t")
                for dc in range(2):
                    for sc_ in range(2):
                        tp = K.ps.tile([128, 128], BF, tag="small", bufs=1)
                        nc.tensor.transpose(
                            tp[:], h2[:, 256 * dc + 128 * sc_:256 * dc + 128 * sc_ + 128],
                            K.ident[:])
                        nc.vector.tensor_copy(
                            h2t[:, 256 * sc_ + 128 * dc:256 * sc_ + 128 * dc + 128], tp[:])
                for j in range(NCORES):
                    dmae = nc.sync if j % 2 == 0 else nc.scalar
                    dmae.dma_start(
                        out=snd2[b][PPC * j + r, :, :],
                        in_=h2t[32 * (j % 4):32 * (j % 4) + 32,
                                256 * (j // 4):256 * (j // 4) + 256])

            pendb = []
            for b in range(B if "B" in PHASES else 0):
                for r in range(PPC):
                    pendb.append(b_stage1(b, r))
                    if len(pendb) > 2:
                        b_stage2(pendb.pop(0))
                while pendb:
                    b_stage2(pendb.pop(0))
                nc.gpsimd.collective_compute(
                    "AllToAll", mybir.AluOpType.bypass,
                    replica_groups=[list(range(NCORES))],
                    ins=[snd2[b].opt()], outs=[rcv2[b].opt()])

            # ====== PHASE C: merge corrections + rmsnorm + FFN (fused) ===
            # process 2 seqs per iteration for 512-wide FFN streams
            seqs = [(b, n) for b in range(B) for n in range(NLOC)]
            for i0 in range(0, len(seqs) if "C" in PHASES else 0, 2):
                pair = seqs[i0:i0 + 2]
                xn2p = K.sb.tile([128, 1024], BF, tag="xn2")
                for s, (b, n) in enumerate(pair):
                    hti = hst[b * NLOC + n]
                    if "B" in PHASES:
                        corr = K.sb.tile([128, 2, TPP], BF, tag="corr")
                        nc.sync.dma_start(out=corr[:],
                                          in_=rcv2[b][0:TPP, n, :], transpose=True)
                        for dc in range(2):
                            nc.vector.tensor_copy(
                                hti[:, 256 * dc:256 * dc + 240:8],
                                corr[:, dc, 0:30])
                            nc.vector.tensor_copy(
                                hti[:, 256 * dc + 240:256 * dc + 256],
                                corr[:, dc, 30:TP])
                    xn2 = _emit_rmsnorm(nc, K, hti, "c")
                    for dc in range(2):
                        nc.vector.tensor_copy(
                            xn2p[:, 512 * dc + 256 * s:512 * dc + 256 * s + 256],
                            xn2[:, 256 * dc:256 * dc + 256])
                # gsb: [128, 4096] bf16, col = 512*fc + 256*s + t
                gsb = K.sb.tile([128, 4096], BF, tag="gsb", bufs=1)
                for fp_ in range(4):
                    fps = K.ps.tile([128, 1024], F32, tag="sc", bufs=2)
                    for sub in range(2):
                        fc = 2 * fp_ + sub
                        for dc in range(2):
                            nc.tensor.matmul(
                                fps[:, 512 * sub:512 * sub + 512],
                                w1[:, dc, 128 * fc:128 * fc + 128],
                                xn2p[:, 512 * dc:512 * dc + 512],
                                start=(dc == 0), stop=(dc == 1))
                    for sub in range(2):
                        fc = 2 * fp_ + sub
                        nc.scalar.activation(
                            out=gsb[:, 512 * fc:512 * fc + 512],
                            in_=fps[:, 512 * sub:512 * sub + 512],
                            func=AF.Gelu_apprx_tanh, bias=b1s[:, fc:fc + 1])
                # ffo: [128, 1024], col = 512*oc + 256*s + t
                ffo = K.ps.tile([128, 1024], F32, tag="sc", bufs=2)
                for oc in range(2):
                    for fc in range(8):
                        nc.tensor.matmul(ffo[:, 512 * oc:512 * oc + 512],
                                         w2[:, fc, 128 * oc:128 * oc + 128],
                                         gsb[:, 512 * fc:512 * fc + 512],
                                         start=(fc == 0), stop=(fc == 7))
                fin = K.sb.tile([128, 1024], BF, tag="fin")
                for s, (b, n) in enumerate(pair):
                    hti = hst[b * NLOC + n]
                    for dc in range(2):
                        nc.vector.tensor_add(
                            fin[:, 512 * dc + 256 * s:512 * dc + 256 * s + 256],
                            ffo[:, 512 * dc + 256 * s:512 * dc + 256 * s + 256],
                            hti[:, 256 * dc:256 * dc + 256])
                for s, (b, n) in enumerate(pair):
                    for dc in range(2):
                        nc.sync.dma_start(
                            out=outfm[b, n][128 * dc:128 * dc + 128, :],
                            in_=fin[:, 512 * dc + 256 * s:512 * dc + 256 * s + 256])
    nc.compile()
    return nc


_CACHED = None


def _get_nc():
    global _CACHED
    if _CACHED is None:
        nc = bacc.Bacc("TRN2", target_bir_lowering=False, debug=False,
                       num_devices=NCORES)
        _CACHED = build(nc)
    return _CACHED


def _host_inputs(h, stock_mask, norm_t_w, norm_s_w, norm_ff_w,
                 Wq_t, Wk_t, Wv_t, Wo_t, Wq_s, Wk_s, Wv_s, Wo_s,
                 W1, b1, W2, b2):
    import ml_dtypes
    bf = ml_dtypes.bfloat16
    f = np.float32
    sc = 1.0 / np.sqrt(HDIM)
    wq_t = (norm_t_w[:, None] * Wq_t * sc).astype(bf)
    wk_t = (norm_t_w[:, None] * Wk_t).astype(bf)
    wv_t = (norm_t_w[:, None] * Wv_t).astype(bf)
    wq_s = (norm_s_w[:, None] * Wq_s * sc).astype(bf)
    wk_s = (norm_s_w[:, None] * Wk_s).astype(bf)
    wv_s = (norm_s_w[:, None] * Wv_s).astype(bf)
    w1 = (norm_ff_w[:, None] * W1).astype(bf)
    mask_bias = ((stock_mask.astype(np.float32) - 1.0) * 1e4).astype(f)  # [B,N]
    bP = np.ascontiguousarray(mask_bias.reshape(B, 2, 128))
    in_maps = []
    for i in range(NCORES):
        hfm = np.ascontiguousarray(
            h[:, i * NLOC:(i + 1) * NLOC].transpose(0, 1, 3, 2)).astype(bf)
        in_maps.append({
            "hfm": hfm, "biasP": bP,
            "wq_t": wq_t, "wk_t": wk_t, "wv_t": wv_t,
            "wo_t": Wo_t.astype(bf),
            "wq_s": wq_s, "wk_s": wk_s, "wv_s": wv_s,
            "wo_s": Wo_s.astype(bf),
            "w1": w1, "w2": W2.astype(bf),
            "b1": b1.astype(f),
        })
    return in_maps


def kernel(**inputs):
    inputs = {k: np.asarray(v) for k, v in inputs.items()}
    nc = _get_nc()
    in_maps = _host_inputs(**inputs)
    res = run_bass_kernel_spmd(nc, in_maps, list(range(NCORES)))
    out = np.empty((B, N, T, D), np.float32)
    for i in range(NCORES):
        out[:, i * NLOC:(i + 1) * NLOC] = \
            res.results[i]["outfm"].astype(np.float32).transpose(0, 1, 3, 2)
    out += inputs["b2"].astype(np.float32)[None, None, None, :]
    return out


if __name__ == "__main__":
    rng = np.random.default_rng(0)
    h = rng.normal(size=(B, N, T, D)).astype(np.float32)
    out = kernel(
        h=h, stock_mask=np.ones((B, N), np.int32),
        norm_t_w=np.ones(D, np.float32), norm_s_w=np.ones(D, np.float32),
        norm_ff_w=np.ones(D, np.float32),
        Wq_t=rng.normal(size=(D, D)).astype(np.float32) * 0.02,
        Wk_t=rng.normal(size=(D, D)).astype(np.float32) * 0.02,
        Wv_t=rng.normal(size=(D, D)).astype(np.float32) * 0.02,
        Wo_t=rng.normal(size=(D, D)).astype(np.float32) * 0.02,
        Wq_s=rng.normal(size=(D, D)).astype(np.float32) * 0.02,
        Wk_s=rng.normal(size=(D, D)).astype(np.float32) * 0.02,
        Wv_s=rng.normal(size=(D, D)).astype(np.float32) * 0.02,
        Wo_s=rng.normal(size=(D, D)).astype(np.float32) * 0.02,
        W1=rng.normal(size=(D, FF)).astype(np.float32) * 0.02,
        b1=np.zeros(FF, np.float32),
        W2=rng.normal(size=(FF, D)).astype(np.float32) * 0.02,
        b2=np.zeros(D, np.float32),
    )
    print("out", out.shape, out.dtype, np.abs(out).max())


# revision 12
# speedup vs baseline: 2.3474x; 1.1222x over previous
"""DualAxisBlock Trainium2 kernel: time-attention + sparse-T stock-attention +
FFN, sharded over 8 NeuronCores (stocks split 32/core, stock-attn pairs
redistributed via b-split AllToAlls).

v2: bf16 matmul datapath (4x PE), DVE Newton-rsqrt rmsnorm (no act-table
thrash), fast-approx reciprocal, SBUF-resident residual stream (no DRAM
round trip between phases), DMA-transpose for stock-attn layout changes.

Self-contained: builds a Bass/Tile SPMD program, shards the full inputs on the
host, runs on cores 0-7, reassembles the full output.
"""
import sys

sys.path.insert(0, "/opt/trn_rl_repo")

import os

import numpy as np

import concourse.bass as bass
import concourse.mybir as mybir
import concourse.tile as tile
from concourse import bacc
from concourse.bass_utils import run_bass_kernel_spmd
from concourse.masks import make_identity

F32 = mybir.dt.float32
BF = mybir.dt.bfloat16
U32 = mybir.dt.uint32
AF = mybir.ActivationFunctionType
ALU = mybir.AluOpType

B, N, T, D = 2, 256, 256, 256
HEADS, HDIM, FF = 8, 32, 1024
NCORES = 8
NLOC = N // NCORES            # 32 stocks per core
TP = 46                       # active timesteps per batch
TPP = 48                      # padded to 48 (6 rows per core chunk)
PPC = TPP // NCORES           # 6 pairs per core per batch block
RSQRT_MAGIC = 0x5F3759DF
PHASES = os.environ.get("KPH", "ABC")     # debug: subset of phases to emit


def _emit_rmsnorm(nc, K, x, tag):
    """x: [128,512] bf16 FM (col=256*dc+t). Returns xn bf16 [128,512].
    rstd via fast-inverse-sqrt seed + 1 Newton iteration (DVE+Pool only;
    avoids scalar-engine act-table loads)."""
    sq = K.sb.tile([128, 512], BF, tag="sq")
    nc.vector.tensor_mul(sq[:], x[:], x[:])
    ss = K.ps.tile([128, 256], F32, tag="small", bufs=1)
    for dc in range(2):
        nc.tensor.matmul(ss[:], K.ones_sc[:], sq[:, 256 * dc:256 * dc + 256],
                         start=(dc == 0), stop=(dc == 1))
    # ss = mean(x^2) per token (broadcast over partitions)
    u = K.sb.tile([128, 256], U32, tag="nru")
    nc.vector.tensor_scalar(u[:], ss[:].bitcast(U32), 1, None,
                            ALU.logical_shift_right)
    y0 = K.sb.tile([128, 256], F32, tag="nry")
    nc.gpsimd.tensor_tensor(out=y0[:].bitcast(U32), in0=K.magic[:], in1=u[:],
                            op=ALU.subtract)
    a = K.sb.tile([128, 256], F32, tag="nra")
    nc.gpsimd.tensor_mul(a[:], y0[:], y0[:])
    bb = K.sb.tile([128, 256], F32, tag="nrb")
    nc.vector.tensor_mul(bb[:], ss[:], a[:])
    dd = K.sb.tile([128, 256], F32, tag="nrd")
    nc.gpsimd.tensor_scalar(dd[:], bb[:], -0.5, 1.5, ALU.mult, ALU.add)
    rstd = K.sb.tile([128, 256], BF, tag="rstd")
    nc.gpsimd.tensor_mul(rstd[:], y0[:], dd[:])
    xn = K.sb.tile([128, 512], BF, tag="xn")
    for dc in range(2):
        nc.vector.tensor_mul(xn[:, 256 * dc:256 * dc + 256],
                             x[:, 256 * dc:256 * dc + 256], rstd[:])
    return xn


def _emit_proj_fm(nc, K, xn, w, tag, engine):
    """q/k projection: out bf16 FM [128,512] (col = 256*oc + t)."""
    ps = K.ps.tile([128, 512], F32, tag="proj", bufs=2)
    for oc in range(2):
        for dc in range(2):
            nc.tensor.matmul(ps[:, 256 * oc:256 * oc + 256],
                             w[:, dc, 128 * oc:128 * oc + 128],
                             xn[:, 256 * dc:256 * dc + 256],
                             start=(dc == 0), stop=(dc == 1))
    out = K.sb.tile([128, 512], BF, tag=tag, bufs=4)
    if engine == "scalar":
        nc.scalar.copy(out=out[:], in_=ps[:])
    elif engine == "gpsimd":
        nc.gpsimd.tensor_copy(out[:], ps[:])
    else:
        nc.vector.tensor_copy(out[:], ps[:])
    return out


def _emit_proj_tm(nc, K, xn, w, tag, engine):
    """v projection token-major: out bf16 [128,512] (col = 256*tokc + dout)."""
    ps = K.ps.tile([128, 512], F32, tag="proj", bufs=2)
    for tc_ in range(2):
        for dc in range(2):
            nc.tensor.matmul(ps[:, 256 * tc_:256 * tc_ + 256],
                             xn[:, 256 * dc + 128 * tc_:256 * dc + 128 * tc_ + 128],
                             w[:, dc, :],
                             start=(dc == 0), stop=(dc == 1))
    out = K.sb.tile([128, 512], BF, tag=tag, bufs=4)
    if engine == "scalar":
        nc.scalar.copy(out=out[:], in_=ps[:])
    elif engine == "gpsimd":
        nc.gpsimd.tensor_copy(out[:], ps[:])
    else:
        nc.vector.tensor_copy(out[:], ps[:])
    return out


def _emit_attn_core(nc, K, q, k, v, bias_ap_fn):
    """Full softmax attention over 256 tokens, 8 heads, bf16 datapath.
    q,k FM bf16 [128,512]; v TM bf16 [128,512]. Returns o_sb bf16 [128,512]
    (col=256*g+t, head 4g+j at partitions 32j). bias_ap_fn(c)->[128,1]|None."""
    S = K.sb.tile([128, 4096], BF, tag="S", bufs=2)
    Sv = S[:].rearrange("p (h c t) -> p h c t", h=8, c=2)
    Ziv = K.sb.tile([128, 512], F32, tag="Ziv")
    o_sb = K.sb.tile([128, 512], BF, tag="osb")
    for hg in range(4):               # half-groups: heads 2hg, 2hg+1
        sc = K.ps.tile([128, 1024], F32, tag="sc", bufs=2)
        for j2 in range(2):
            h = 2 * hg + j2
            j = h % 4
            g = h // 4
            for c in range(2):
                qb = 256 * g
                nc.tensor.matmul(
                    sc[:, 512 * j2 + 256 * c:512 * j2 + 256 * c + 256],
                    k[32 * j:32 * j + 32, qb + 128 * c:qb + 128 * c + 128],
                    q[32 * j:32 * j + 32, qb:qb + 256],
                    start=True, stop=True, tile_position=(32 * j, 0))
        if bias_ap_fn is None:
            nc.scalar.activation(out=S[:, 1024 * hg:1024 * hg + 1024],
                                 in_=sc[:], func=AF.Exp)
        else:
            scv = sc[:].rearrange("p (j2 c t) -> p j2 c t", j2=2, c=2)
            for c in range(2):
                nc.scalar.activation(out=Sv[:, 2 * hg:2 * hg + 2, c, :],
                                     in_=scv[:, :, c, :], func=AF.Exp,
                                     bias=bias_ap_fn(c))
    for g in range(2):
        zb = K.ps.tile([128, 256], F32, tag="small", bufs=1)
        for j in range(4):
            for c in range(2):
                nc.tensor.matmul(zb[32 * j:32 * j + 32, :], K.ones[:, 0:32],
                                 Sv[:, 4 * g + j, c, :],
                                 start=(c == 0), stop=(c == 1),
                                 tile_position=(0, 32 * j))
        nc.vector.reciprocal_approx_fast(out=Ziv[:, 256 * g:256 * g + 256],
                                         in_=zb[:])
        op = K.ps.tile([128, 256], F32, tag="ops")
        for j in range(4):
            for c in range(2):
                h = 4 * g + j
                nc.tensor.matmul(op[32 * j:32 * j + 32, :],
                                 v[:, 256 * c + 32 * h:256 * c + 32 * h + 32],
                                 Sv[:, h, c, :],
                                 start=(c == 0), stop=(c == 1),
                                 tile_position=(0, 32 * j))
        nc.vector.tensor_mul(o_sb[:, 256 * g:256 * g + 256],
                             op[:], Ziv[:, 256 * g:256 * g + 256])
    return o_sb


def _emit_wo_resid(nc, K, o_sb, wo, x, h_out):
    """h_out[:] = x + o @ Wo (bf16)."""
    ps = K.ps.tile([128, 512], F32, tag="proj", bufs=2)
    for oc in range(2):
        for dmc in range(2):
            nc.tensor.matmul(ps[:, 256 * oc:256 * oc + 256],
                             wo[:, dmc, 128 * oc:128 * oc + 128],
                             o_sb[:, 256 * dmc:256 * dmc + 256],
                             start=(dmc == 0), stop=(dmc == 1))
    for c in range(2):
        nc.vector.tensor_add(h_out[:, 256 * c:256 * c + 256],
                             x[:, 256 * c:256 * c + 256],
                             ps[:, 256 * c:256 * c + 256])


class _K:
    pass


def build(nc):
    hfm = nc.declare_dram_parameter("hfm", [B, NLOC, D, T], BF, isOutput=False)
    biasP = nc.declare_dram_parameter("biasP", [B, 2, 128], F32, isOutput=False)
    wnames = ["wq_t", "wk_t", "wv_t", "wo_t", "wq_s", "wk_s", "wv_s", "wo_s"]
    wext = {w: nc.declare_dram_parameter(w, [D, D], BF, isOutput=False)
            for w in wnames}
    w1e = nc.declare_dram_parameter("w1", [D, FF], BF, isOutput=False)
    w2e = nc.declare_dram_parameter("w2", [FF, D], BF, isOutput=False)
    b1e = nc.declare_dram_parameter("b1", [FF], F32, isOutput=False)
    outfm = nc.declare_dram_parameter("outfm", [B, NLOC, D, T], BF, isOutput=True)

    K = _K()
    with tile.TileContext(nc) as tc:
        import contextlib
        with contextlib.ExitStack() as ctx:
            K.sb = ctx.enter_context(tc.tile_pool(name="sb", bufs=2))
            K.single = ctx.enter_context(tc.tile_pool(name="single", bufs=1))
            K.ps = ctx.enter_context(tc.tile_pool(name="ps", bufs=1, space="PSUM"))
            dram = ctx.enter_context(tc.tile_pool(name="dram", bufs=1, space="DRAM"))

            snd1 = [dram.tile([TPP, NLOC, D], BF, tag=f"snd1{b}", name=f"snd1{b}")
                    for b in range(B)]
            rcv1 = [dram.tile([TPP, NLOC, D], BF, tag=f"rcv1{b}", name=f"rcv1{b}")
                    for b in range(B)]
            snd2 = [dram.tile([TPP, NLOC, D], BF, tag=f"snd2{b}", name=f"snd2{b}")
                    for b in range(B)]
            rcv2 = [dram.tile([TPP, NLOC, D], BF, tag=f"rcv2{b}", name=f"rcv2{b}")
                    for b in range(B)]

            # ---- constants / weights to SBUF ----
            K.ones = K.single.tile([128, 128], BF, tag="ones")
            nc.gpsimd.memset(K.ones[:], 1.0)
            K.ones_sc = K.single.tile([128, 128], BF, tag="ones_sc")
            nc.gpsimd.memset(K.ones_sc[:], 1.0 / D)
            K.ident = K.single.tile([128, 128], BF, tag="ident")
            make_identity(nc, K.ident[:])
            K.magic = K.single.tile([128, 256], U32, tag="magic")
            nc.gpsimd.memset(K.magic[:], RSQRT_MAGIC)
            zer = K.single.tile([128, D], BF, tag="zer")
            nc.gpsimd.memset(zer[:], 0.0)

            wsb = {}
            for w in wnames:
                wsb[w] = K.single.tile([128, 2, D], BF, tag=w, name=w)
                nc.sync.dma_start(out=wsb[w][:],
                                  in_=wext[w][:].rearrange("(c p) x -> p c x", p=128))
            w1 = K.single.tile([128, 2, FF], BF, tag="w1")
            nc.sync.dma_start(out=w1[:], in_=w1e[:].rearrange("(c p) x -> p c x", p=128))
            w2 = K.single.tile([128, 8, D], BF, tag="w2")
            nc.sync.dma_start(out=w2[:], in_=w2e[:].rearrange("(c p) x -> p c x", p=128))
            b1s = K.single.tile([128, 8], F32, tag="b1s")
            nc.sync.dma_start(out=b1s[:], in_=b1e[:].rearrange("(c p) -> p c", p=128))
            bps = K.single.tile([128, B, 2], F32, tag="bps")
            nc.sync.dma_start(out=bps[:], in_=biasP[:].rearrange("b c p -> p b c"))

            # persistent residual stream: one bf16 tile per local sequence
            hst = [K.single.tile([128, 512], BF, tag=f"h{i}", name=f"h{i}")
                   for i in range(B * NLOC)]

            # zero the pad rows (t rows 46,47) of each snd1 block
            for b in range(B):
                for pad in range(TP, TPP):
                    nc.sync.dma_start(out=snd1[b][pad], in_=zer[0:NLOC, :])

            # ================= PHASE A: time attention (3-stage pipe) ====
            def a_stage1(b, n):
                x = K.sb.tile([128, 512], BF, tag="x", bufs=4)
                dmae = nc.sync if n % 2 == 0 else nc.scalar
                dmae.dma_start(
                    out=x[:].rearrange("p (c t) -> p c t", c=2),
                    in_=hfm[b, n].rearrange("(c p) t -> p c t", p=128))
                xn = _emit_rmsnorm(nc, K, x, "a")
                q = _emit_proj_fm(nc, K, xn, wsb["wq_t"], "q", "scalar")
                k = _emit_proj_fm(nc, K, xn, wsb["wk_t"], "k", "scalar")
                v = _emit_proj_tm(nc, K, xn, wsb["wv_t"], "v", "vector")
                return (b, n, x, q, k, v)

            def a_stage2(st):
                b, n, x, q, k, v = st
                hti = hst[b * NLOC + n]
                o = _emit_attn_core(nc, K, q, k, v, None)
                _emit_wo_resid(nc, K, o, wsb["wo_t"], x, hti)
                # extract active-t columns token-major for the AllToAll
                tp = K.ps.tile([128, 128], BF, tag="small", bufs=1)
                h1a = K.sb.tile([128, 256], BF, tag="h1a")
                for dc in range(2):
                    nc.tensor.transpose(tp[64 * dc:64 * dc + 30, :],
                                        hti[:, 256 * dc:256 * dc + 240:8],
                                        K.ident[:], tile_position=(0, 64 * dc))
                    nc.tensor.transpose(tp[64 * dc + 32:64 * dc + 48, :],
                                        hti[:, 256 * dc + 240:256 * dc + 256],
                                        K.ident[:], tile_position=(0, 64 * dc + 32))
                    nc.vector.tensor_copy(h1a[0:30, 128 * dc:128 * dc + 128],
                                          tp[64 * dc:64 * dc + 30, :])
                    nc.vector.tensor_copy(h1a[32:48, 128 * dc:128 * dc + 128],
                                          tp[64 * dc + 32:64 * dc + 48, :])
                nc.sync.dma_start(out=snd1[b][0:30, n, :], in_=h1a[0:30, :])
                nc.scalar.dma_start(out=snd1[b][30:TP, n, :], in_=h1a[32:48, :])
                if "C" not in PHASES:
                    nc.sync.dma_start(
                        out=outfm[b, n].rearrange("(c p) t -> p c t", p=128),
                        in_=hti[:].rearrange("p (c t) -> p c t", c=2))

            pend = []
            for b in range(B):
                for n in range(NLOC):
                    pend.append(a_stage1(b, n))
                    if len(pend) > 2:
                        a_stage2(pend.pop(0))
                if "B" in PHASES:
                    while pend:
                        a_stage2(pend.pop(0))
                    nc.gpsimd.collective_compute(
                        "AllToAll", mybir.AluOpType.bypass,
                        replica_groups=[list(range(NCORES))],
                        ins=[snd1[b].opt()], outs=[rcv1[b].opt()])
            while pend:
                a_stage2(pend.pop(0))

            # ================= PHASE B: stock attention =================
            # local pair (b, r): global t-row = PPC*core + r; rcv1[b] rows
            # {6j + r} hold stocks [32j,32j+32) token-major.
            def b_stage1(b, r):
                xsf = K.sb.tile([128, 2, 256], BF, tag="xsf", bufs=4)
                for j in range(NCORES):
                    dmae = nc.sync if j % 2 == 0 else nc.scalar
                    dmae.dma_start(
                        out=xsf[:, :, 32 * j:32 * j + 32],
                        in_=rcv1[b][PPC * j + r], transpose=True)
                xsfv = xsf[:].rearrange("p c s -> p (c s)")
                xn = _emit_rmsnorm(nc, K, xsfv, "b")
                q = _emit_proj_fm(nc, K, xn, wsb["wq_s"], "q", "scalar")
                k = _emit_proj_fm(nc, K, xn, wsb["wk_s"], "k", "scalar")
                v = _emit_proj_tm(nc, K, xn, wsb["wv_s"], "v", "vector")
                return (b, r, xsfv, q, k, v)

            def b_stage2(st):
                b, r, xsf, q, k, v = st
                o = _emit_attn_core(nc, K, q, k, v,
                                    lambda c: bps[:, b, c:c + 1])
                h2 = K.sb.tile([128, 512], BF, tag="h2")
                _emit_wo_resid(nc, K, o, wsb["wo_s"], xsf, h2)
                h2t = K.sb.tile([128, 512], BF, tag="h2t")
                for dc in range(2):
                    for sc_ in range(2):
                        tp = K.ps.tile([128, 128], BF, tag="small", bufs=1)
                        nc.tensor.transpose(
                            tp[:], h2[:, 256 * dc + 128 * sc_:256 * dc + 128 * sc_ + 128],
                            K.ident[:])
                        nc.vector.tensor_copy(
                            h2t[:, 256 * sc_ + 128 * dc:256 * sc_ + 128 * dc + 128], tp[:])
                for j in range(NCORES):
                    dmae = nc.sync if j % 2 == 0 else nc.scalar
                    dmae.dma_start(
                        out=snd2[b][PPC * j + r, :, :],
                        in_=h2t[32 * (j % 4):32 * (j % 4) + 32,
                                256 * (j // 4):256 * (j // 4) + 256])

            pendb = []
            for b in range(B if "B" in PHASES else 0):
                for r in range(PPC):
                    pendb.append(b_stage1(b, r))
                    if len(pendb) > 2:
                        b_stage2(pendb.pop(0))
                while pendb:
                    b_stage2(pendb.pop(0))
                nc.gpsimd.collective_compute(
                    "AllToAll", mybir.AluOpType.bypass,
                    replica_groups=[list(range(NCORES))],
                    ins=[snd2[b].opt()], outs=[rcv2[b].opt()])

            # ====== PHASE C: merge corrections + rmsnorm + FFN (fused) ===
            # 2-stage pipeline over pairs of seqs (512-wide FFN streams)
            seqs = [(b, n) for b in range(B) for n in range(NLOC)]

            def c_stage1(i0):
                pair = seqs[i0:i0 + 2]
                xn2p = K.sb.tile([128, 1024], BF, tag="xn2", bufs=4)
                for s, (b, n) in enumerate(pair):
                    hti = hst[b * NLOC + n]
                    if "B" in PHASES:
                        corr = K.sb.tile([128, 2, TPP], BF, tag="corr", bufs=4)
                        dmae = nc.sync if (i0 + s) % 2 == 0 else nc.scalar
                        dmae.dma_start(out=corr[:],
                                       in_=rcv2[b][0:TPP, n, :], transpose=True)
                        for dc in range(2):
                            nc.vector.tensor_copy(
                                hti[:, 256 * dc:256 * dc + 240:8],
                                corr[:, dc, 0:30])
                            nc.vector.tensor_copy(
                                hti[:, 256 * dc + 240:256 * dc + 256],
                                corr[:, dc, 30:TP])
                    xn2 = _emit_rmsnorm(nc, K, hti, "c")
                    for dc in range(2):
                        nc.vector.tensor_copy(
                            xn2p[:, 512 * dc + 256 * s:512 * dc + 256 * s + 256],
                            xn2[:, 256 * dc:256 * dc + 256])
                return (i0, xn2p)

            def c_stage2(st):
                i0, xn2p = st
                pair = seqs[i0:i0 + 2]
                # gsb: [128, 4096] bf16, col = 512*fc + 256*s + t
                gsb = K.sb.tile([128, 4096], BF, tag="gsb", bufs=2)
                for fp_ in range(4):
                    fps = K.ps.tile([128, 1024], F32, tag="sc", bufs=2)
                    for sub in range(2):
                        fc = 2 * fp_ + sub
                        for dc in range(2):
                            nc.tensor.matmul(
                                fps[:, 512 * sub:512 * sub + 512],
                                w1[:, dc, 128 * fc:128 * fc + 128],
                                xn2p[:, 512 * dc:512 * dc + 512],
                                start=(dc == 0), stop=(dc == 1))
                    for sub in range(2):
                        fc = 2 * fp_ + sub
                        nc.scalar.activation(
                            out=gsb[:, 512 * fc:512 * fc + 512],
                            in_=fps[:, 512 * sub:512 * sub + 512],
                            func=AF.Gelu_apprx_tanh, bias=b1s[:, fc:fc + 1])
                # ffo: [128, 1024], col = 512*oc + 256*s + t
                ffo = K.ps.tile([128, 1024], F32, tag="sc", bufs=2)
                for oc in range(2):
                    for fc in range(8):
                        nc.tensor.matmul(ffo[:, 512 * oc:512 * oc + 512],
                                         w2[:, fc, 128 * oc:128 * oc + 128],
                                         gsb[:, 512 * fc:512 * fc + 512],
                                         start=(fc == 0), stop=(fc == 7))
                fin = K.sb.tile([128, 1024], BF, tag="fin")
                for s, (b, n) in enumerate(pair):
                    hti = hst[b * NLOC + n]
                    for dc in range(2):
                        nc.vector.tensor_add(
                            fin[:, 512 * dc + 256 * s:512 * dc + 256 * s + 256],
                            ffo[:, 512 * dc + 256 * s:512 * dc + 256 * s + 256],
                            hti[:, 256 * dc:256 * dc + 256])
                for s, (b, n) in enumerate(pair):
                    for dc in range(2):
                        dmae = nc.sync if (s + dc) % 2 == 0 else nc.scalar
                        dmae.dma_start(
                            out=outfm[b, n][128 * dc:128 * dc + 128, :],
                            in_=fin[:, 512 * dc + 256 * s:512 * dc + 256 * s + 256])

            seqs2 = seqs
            pendc = []
            for i0 in range(0, len(seqs2) if "C" in PHASES else 0, 2):
                pendc.append(c_stage1(i0))
                if len(pendc) > 1:
                    c_stage2(pendc.pop(0))
            while pendc:
                c_stage2(pendc.pop(0))
    nc.compile()
    return nc


_CACHED = None


def _get_nc():
    global _CACHED
    if _CACHED is None:
        nc = bacc.Bacc("TRN2", target_bir_lowering=False, debug=False,
                       num_devices=NCORES)
        _CACHED = build(nc)
    return _CACHED


def _host_inputs(h, stock_mask, norm_t_w, norm_s_w, norm_ff_w,
                 Wq_t, Wk_t, Wv_t, Wo_t, Wq_s, Wk_s, Wv_s, Wo_s,
                 W1, b1, W2, b2):
    import ml_dtypes
    bf = ml_dtypes.bfloat16
    f = np.float32
    sc = 1.0 / np.sqrt(HDIM)
    wq_t = (norm_t_w[:, None] * Wq_t * sc).astype(bf)
    wk_t = (norm_t_w[:, None] * Wk_t).astype(bf)
    wv_t = (norm_t_w[:, None] * Wv_t).astype(bf)
    wq_s = (norm_s_w[:, None] * Wq_s * sc).astype(bf)
    wk_s = (norm_s_w[:, None] * Wk_s).astype(bf)
    wv_s = (norm_s_w[:, None] * Wv_s).astype(bf)
    w1 = (norm_ff_w[:, None] * W1).astype(bf)
    mask_bias = ((stock_mask.astype(np.float32) - 1.0) * 1e4).astype(f)  # [B,N]
    bP = np.ascontiguousarray(mask_bias.reshape(B, 2, 128))
    in_maps = []
    for i in range(NCORES):
        hfm = np.ascontiguousarray(
            h[:, i * NLOC:(i + 1) * NLOC].transpose(0, 1, 3, 2)).astype(bf)
        in_maps.append({
            "hfm": hfm, "biasP": bP,
            "wq_t": wq_t, "wk_t": wk_t, "wv_t": wv_t,
            "wo_t": Wo_t.astype(bf),
            "wq_s": wq_s, "wk_s": wk_s, "wv_s": wv_s,
            "wo_s": Wo_s.astype(bf),
            "w1": w1, "w2": W2.astype(bf),
            "b1": b1.astype(f),
        })
    return in_maps


def kernel(**inputs):
    inputs = {k: np.asarray(v) for k, v in inputs.items()}
    nc = _get_nc()
    in_maps = _host_inputs(**inputs)
    res = run_bass_kernel_spmd(nc, in_maps, list(range(NCORES)))
    out = np.empty((B, N, T, D), np.float32)
    for i in range(NCORES):
        out[:, i * NLOC:(i + 1) * NLOC] = \
            res.results[i]["outfm"].astype(np.float32).transpose(0, 1, 3, 2)
    out += inputs["b2"].astype(np.float32)[None, None, None, :]
    return out


if __name__ == "__main__":
    rng = np.random.default_rng(0)
    h = rng.normal(size=(B, N, T, D)).astype(np.float32)
    out = kernel(
        h=h, stock_mask=np.ones((B, N), np.int32),
        norm_t_w=np.ones(D, np.float32), norm_s_w=np.ones(D, np.float32),
        norm_ff_w=np.ones(D, np.float32),
        Wq_t=rng.normal(size=(D, D)).astype(np.float32) * 0.02,
        Wk_t=rng.normal(size=(D, D)).astype(np.float32) * 0.02,
        Wv_t=rng.normal(size=(D, D)).astype(np.float32) * 0.02,
        Wo_t=rng.normal(size=(D, D)).astype(np.float32) * 0.02,
        Wq_s=rng.normal(size=(D, D)).astype(np.float32) * 0.02,
        Wk_s=rng.normal(size=(D, D)).astype(np.float32) * 0.02,
        Wv_s=rng.normal(size=(D, D)).astype(np.float32) * 0.02,
        Wo_s=rng.normal(size=(D, D)).astype(np.float32) * 0.02,
        W1=rng.normal(size=(D, FF)).astype(np.float32) * 0.02,
        b1=np.zeros(FF, np.float32),
        W2=rng.normal(size=(FF, D)).astype(np.float32) * 0.02,
        b2=np.zeros(D, np.float32),
    )
    print("out", out.shape, out.dtype, np.abs(out).max())
